# revision 3
# baseline (speedup 1.0000x reference)
"""Trainium2 Bass kernel for nn_Dist_Conv2D_Dense (Chebyshev-distance "conv").

Computation (per batch b, output channel co, position (h, w)):
    out[b, co, h, w] = max_{cin, kh, kw} |x[b, cin, h+kh-1, w+kw-1] - weights[co, cin, kh, kw]| + bias[co]
with replicate ("edge") padding, for x (8, 16, 64, 64), weights (32, 16, 3, 3).

Sharding: data-parallel over batch, B=8 -> one batch element per NeuronCore.

SCHEME "v2" (default) - 3-engine pipeline, rebalanced + batched tree:
  * TensorE produces (x - w) diffs for the first NB channels via a selector
    matmul (stationary lhsT = 73 rows: 72 pre-shifted input planes + ones row;
    moving columns have a 1 at row d and -w[co,d] in the ones row).
  * ScalarE drains PSUM with Abs, casting to fp16 into a unified staged tile.
  * VectorE subtracts the remaining ND channels directly (fp16 2x broadcast),
    writing RAW diffs into the same staged tile.
  * VectorE max-tree over all 32 channels, batched TWO row-pairs per
    instruction to amortize the ~58-cycle DVE instruction overhead; level 1
    uses op=abs_max which both combines the halves and absorbs the abs for
    the DVE-produced channels (ACT-drained values are nonneg, so abs_max==max).
  * DMA staged so the first matmul's inputs (sel-A chunk 0 + x quarter 0)
    land first, splitting issue across the two HWDGE queues (sync/scalar).

SCHEME "hybrid": previous 141.5us baseline kept for reference/fallback.
"""

import numpy as np
from contextlib import ExitStack

# Problem constants (hardcoded per spec)
B, CIN, H, W = 8, 16, 64, 64
COUT, K = 32, 3
N_CORES = 8
HPAD = H + 2  # 66
D = CIN * K * K  # 144
DH = D // 2  # 72, half-window length

SCHEME = "v2"  # "v2" | "hybrid"
# v2 channel split: NB channels PE->ACT, ND = 32-NB channels DVE-subtract
NB2 = 25
ND2 = 32 - NB2
L1_ABSMAX = False  # abs_max has no TRN2 encoding (walrus rejects); use int16 mask
# hybrid params (legacy)
NB = 26
ND = 6

_PROGRAM_CACHE = {}
LAST_RESULTS = None  # stashed BassKernelResults for the test harness


# ------------------------------------------------------------------ v2 scheme

def _build_program_v2():
    import concourse.bacc as bacc
    import concourse.mybir as mybir
    from concourse.alu_op_type import AluOpType
    from concourse.tile import TileContext

    F16, F32, I16 = mybir.dt.float16, mybir.dt.float32, mybir.dt.int16
    PCOLS = NB2 * DH  # psum columns per half

    nc = bacc.Bacc(
        "TRN2", target_bir_lowering=False, debug=False, num_devices=N_CORES
    )

    XA0 = 0
    XB0 = XA0 + H * W
    SA0 = XB0 + H * W
    SB0 = SA0 + PCOLS
    X3B0 = SB0 + PCOLS
    WCD0 = X3B0 + 3 * HPAD * CIN
    B0 = WCD0 + ND2 * D
    BLOB = B0 + 32 * COUT
    blob_d = nc.dram_tensor("blob", [128, BLOB], F16, kind="ExternalInput")
    out_d = nc.dram_tensor("out", [128, 32 * COUT], F16, kind="ExternalOutput")

    with TileContext(nc) as tc:
        with (
            tc.tile_pool(name="io", bufs=1) as io_pool,
            tc.tile_pool(name="ps", bufs=2, space="PSUM") as ps_pool,
            tc.tile_pool(name="st", bufs=3) as st_pool,
            tc.tile_pool(name="tr", bufs=2) as tr_pool,
        ):
            blob_t = io_pool.tile([128, BLOB], F16)
            QC = H * W // 8  # 512-column x quarters

            # -- DMA staging: first matmul needs sel-A chunk 0 + xa quarter 0.
            # Alternate the two HWDGE queues (sync / scalar) for concurrency.
            nc.sync.dma_start(out=blob_t[:, SA0 : SA0 + 512], in_=blob_d.ap()[:, SA0 : SA0 + 512])
            nc.scalar.dma_start(out=blob_t[:, XA0 : XA0 + QC], in_=blob_d.ap()[:, XA0 : XA0 + QC])
            nc.sync.dma_start(out=blob_t[:, SA0 + 512 : SB0], in_=blob_d.ap()[:, SA0 + 512 : SB0])
            nc.scalar.dma_start(out=blob_t[:, SB0 : SB0 + 512], in_=blob_d.ap()[:, SB0 : SB0 + 512])
            nc.sync.dma_start(out=blob_t[:, XB0 : XB0 + QC], in_=blob_d.ap()[:, XB0 : XB0 + QC])
            nc.scalar.dma_start(out=blob_t[:, SB0 + 512 : X3B0], in_=blob_d.ap()[:, SB0 + 512 : X3B0])
            # x3b + wcd: needed by the first DVE subtract (tolerates ~1 rp lag)
            nc.sync.dma_start(out=blob_t[:, X3B0:B0], in_=blob_d.ap()[:, X3B0:B0])
            for q in range(1, 8):
                a = XA0 + q * QC
                nc.scalar.dma_start(out=blob_t[:, a : a + QC], in_=blob_d.ap()[:, a : a + QC])
                b = XB0 + q * QC
                nc.sync.dma_start(out=blob_t[:, b : b + QC], in_=blob_d.ap()[:, b : b + QC])
            nc.scalar.dma_start(out=blob_t[:, B0:BLOB], in_=blob_d.ap()[:, B0:BLOB])

            xa_t = blob_t[0 : DH + 1, XA0 : XA0 + H * W]
            xb_t = blob_t[0 : DH + 1, XB0 : XB0 + H * W]
            sa_t = blob_t[0 : DH + 1, SA0 : SA0 + PCOLS]
            sb_t = blob_t[0 : DH + 1, SB0 : SB0 + PCOLS]
            x3b_t = blob_t[:, X3B0 : X3B0 + 3 * HPAD * CIN]
            wcd_t = blob_t[:, WCD0 : WCD0 + ND2 * D]
            bias_t = blob_t[:, B0 : B0 + 32 * COUT]

            # acc columns: (r2, co)
            acc_t = io_pool.tile([128, 32 * COUT], F16)

            xa4 = xa_t.rearrange("k (h w) -> k h w", h=H)
            xb4 = xb_t.rearrange("k (h w) -> k h w", h=H)
            x4 = x3b_t.rearrange("p (kw h c) -> p kw h c", kw=3, h=HPAD)
            w5 = wcd_t.rearrange(
                "p (co kw kh c) -> p co kw kh c", co=ND2, kw=3, kh=3
            )

            def produce(r2, staged):
                """Emit PE+ACT+DVE production of row-pair r2 into staged
                (a [128, 2*32*D] tile; r2&1 selects the half)."""
                r = 2 * r2
                sv = staged[:, :].rearrange(
                    "p (e u j) -> p e u j", e=2, j=D
                )
                for half in range(2):
                    x_t = (xa4 if half == 0 else xb4)[:, r : r + 2, :]  # [73,2,64]
                    s_t = sa_t if half == 0 else sb_t
                    ps_t = ps_pool.tile(
                        [128, PCOLS], F32, tag="ps", name=f"ps{r2}_{half}"
                    )
                    for m0 in range(0, PCOLS, 512):
                        m1 = min(m0 + 512, PCOLS)
                        nc.tensor.matmul(
                            out=ps_t[:, m0:m1],
                            lhsT=x_t,
                            rhs=s_t[:, m0:m1],
                            start=True,
                            stop=True,
                        )
                    nc.scalar.activation(
                        out=sv[:, r2 % 2, 0:NB2, half * DH : (half + 1) * DH],
                        in_=ps_t[:, :].rearrange("p (u j) -> p u j", j=DH),
                        func=mybir.ActivationFunctionType.Abs,
                    )
                # DVE subtract for the last ND2 channels (raw diffs; abs is
                # folded into tree L1's abs_max)
                s5 = sv[:, r2 % 2, NB2:32, :].rearrange(
                    "p co (kw kh c) -> p co kw kh c", kw=3, kh=3
                )
                x5b = (
                    x4[:, :, r : r + 3, :]
                    .unsqueeze(1)
                    .broadcast_to((128, ND2, 3, 3, CIN))
                )
                nc.vector.tensor_tensor(out=s5, in0=x5b, in1=w5, op=AluOpType.subtract)
                if not L1_ABSMAX:
                    nc.vector.tensor_scalar(
                        out=sv[:, r2 % 2, NB2:32, :].bitcast(I16),
                        in0=sv[:, r2 % 2, NB2:32, :].bitcast(I16),
                        scalar1=0x7FFF,
                        scalar2=None,
                        op0=AluOpType.bitwise_and,
                    )

            def emit_tree(pair, staged):
                """Max-tree for row-pairs (2*pair, 2*pair+1), batched in one
                instruction per level over 64 units of 144."""
                s4 = staged[:, :].rearrange("p (u j) -> p u j", j=D)  # u = 64
                t1_t = tr_pool.tile([128, 64 * DH], F16, tag="t1", name=f"t1_{pair}")
                t1 = t1_t[:, :].rearrange("p (u j) -> p u j", j=DH)
                nc.vector.tensor_tensor(
                    out=t1,
                    in0=s4[:, :, 0:DH],
                    in1=s4[:, :, DH:D],
                    op=AluOpType.abs_max if L1_ABSMAX else AluOpType.max,
                )
                t2_t = tr_pool.tile([128, 64 * 36], F16, tag="t2", name=f"t2_{pair}")
                t2 = t2_t[:, :].rearrange("p (u j) -> p u j", j=36)
                nc.vector.tensor_tensor(
                    out=t2, in0=t1[:, :, 0:36], in1=t1[:, :, 36:72], op=AluOpType.max
                )
                t3_t = tr_pool.tile([128, 64 * 18], F16, tag="t3", name=f"t3_{pair}")
                t3 = t3_t[:, :].rearrange("p (u j) -> p u j", j=18)
                nc.vector.tensor_tensor(
                    out=t3, in0=t2[:, :, 0:18], in1=t2[:, :, 18:36], op=AluOpType.max
                )
                t4_t = tr_pool.tile([128, 64 * 9], F16, tag="t4", name=f"t4_{pair}")
                t4 = t4_t[:, :].rearrange("p (u j) -> p u j", j=9)
                nc.vector.tensor_tensor(
                    out=t4, in0=t3[:, :, 0:9], in1=t3[:, :, 9:18], op=AluOpType.max
                )
                nc.vector.tensor_reduce(
                    out=acc_t[:, pair * 64 : (pair + 1) * 64],
                    in_=t4,
                    axis=mybir.AxisListType.X,
                    op=AluOpType.max,
                )
                if pair % 4 == 3:
                    # this output quarter is complete: bias + store now
                    q = pair // 4
                    nc.vector.tensor_tensor(
                        out=acc_t[:, q * 256 : (q + 1) * 256],
                        in0=acc_t[:, q * 256 : (q + 1) * 256],
                        in1=bias_t[:, q * 256 : (q + 1) * 256],
                        op=AluOpType.add,
                    )
                    nc.sync.dma_start(
                        out=out_d.ap()[:, q * 256 : (q + 1) * 256],
                        in_=acc_t[:, q * 256 : (q + 1) * 256],
                    )

            staged_tiles = {}
            for r2 in range(32):
                pair = r2 // 2
                if r2 % 2 == 0:
                    staged_tiles[pair] = st_pool.tile(
                        [128, 2 * 32 * D], F16, tag="stg", name=f"stg{pair}"
                    )
                produce(r2, staged_tiles[pair])
                # software pipeline: tree for pair k emitted after pair k+1's
                # production, so ScalarE has a full pair-window to finish
                if r2 % 2 == 1 and pair >= 1:
                    emit_tree(pair - 1, staged_tiles.pop(pair - 1))
            emit_tree(15, staged_tiles.pop(15))

    nc.compile()
    return nc


def _prep_inputs_v2(x, weights, bias):
    PCOLS = NB2 * DH
    XA0 = 0
    XB0 = XA0 + H * W
    SA0 = XB0 + H * W
    SB0 = SA0 + PCOLS
    X3B0 = SB0 + PCOLS
    WCD0 = X3B0 + 3 * HPAD * CIN
    B0 = WCD0 + ND2 * D
    BLOB = B0 + 32 * COUT

    w_perm = np.ascontiguousarray(weights.transpose(0, 3, 2, 1)).reshape(COUT, D)

    def selector(half):
        s = np.zeros((DH + 1, NB2, DH), dtype=np.float32)
        for j in range(DH):
            s[j, :, j] = 1.0
        s[DH, :, :] = -w_perm[:NB2, half * DH : (half + 1) * DH]
        return s.reshape(DH + 1, PCOLS).astype(np.float16)

    sa = selector(0)
    sb = selector(1)
    wcd = np.broadcast_to(w_perm[NB2:].reshape(1, ND2 * D), (128, ND2 * D))
    biasb = np.broadcast_to(
        np.tile(bias.reshape(COUT), 32)[None, :], (128, 32 * COUT)
    )

    in_maps = []
    for core in range(N_CORES):
        xc = x[core]
        x_pad = np.pad(xc, ((0, 0), (1, 1), (1, 1)), mode="edge")
        planes = np.empty((3, 3, CIN, H, W), dtype=np.float32)  # (kw, kh, cin, h, w)
        for kw in range(3):
            for kh in range(3):
                planes[kw, kh] = x_pad[:, kh : kh + H, kw : kw + W]
        planes = planes.reshape(D, H * W)
        ones = np.ones((1, H * W), dtype=np.float32)
        blob = np.zeros((128, BLOB), dtype=np.float16)
        blob[: DH + 1, XA0 : XA0 + H * W] = np.concatenate([planes[:DH], ones], 0)
        blob[: DH + 1, XB0 : XB0 + H * W] = np.concatenate([planes[DH:], ones], 0)
        blob[: DH + 1, SA0 : SA0 + PCOLS] = sa
        blob[: DH + 1, SB0 : SB0 + PCOLS] = sb
        blob[:, X3B0 : X3B0 + 3 * HPAD * CIN] = _build_x3b_f16(xc)
        blob[:, WCD0 : WCD0 + ND2 * D] = wcd
        blob[:, B0 : B0 + 32 * COUT] = biasb
        in_maps.append({"blob": blob})
    return in_maps


# ------------------------------------------------------------ hybrid scheme

def _build_program_hybrid():
    import concourse.bacc as bacc
    import concourse.mybir as mybir
    from concourse.alu_op_type import AluOpType
    from concourse.tile import TileContext

    F16, F32, I16 = mybir.dt.float16, mybir.dt.float32, mybir.dt.int16
    NC = 32 - NB - ND          # DVE-sub + ACT-abs channels
    NCD = NC + ND              # all DVE-subtracted channels
    PCOLS = NB * DH            # psum columns per half-chunk

    nc = bacc.Bacc(
        "TRN2", target_bir_lowering=False, debug=False, num_devices=N_CORES
    )

    XA0 = 0
    XB0 = XA0 + H * W
    SA0 = XB0 + H * W
    SB0 = SA0 + PCOLS
    X3B0 = SB0 + PCOLS
    WCD0 = X3B0 + 3 * HPAD * CIN
    B0 = WCD0 + NCD * D
    BLOB = B0 + 32 * COUT
    blob_d = nc.dram_tensor("blob", [128, BLOB], F16, kind="ExternalInput")
    out_d = nc.dram_tensor("out", [128, 32 * COUT], F16, kind="ExternalOutput")

    with TileContext(nc) as tc:
        with (
            tc.tile_pool(name="io", bufs=1) as io_pool,
            tc.tile_pool(name="ps", bufs=2, space="PSUM") as ps_pool,
            tc.tile_pool(name="st", bufs=4) as st_pool,
            tc.tile_pool(name="sc", bufs=6) as sc_pool,
            tc.tile_pool(name="tr", bufs=4) as tr_pool,
        ):
            blob_t = io_pool.tile([128, BLOB], F16)
            nc.sync.dma_start(out=blob_t[:, SA0:SB0], in_=blob_d.ap()[:, SA0:SB0])
            QC = H * W // 8
            nc.sync.dma_start(out=blob_t[:, XA0 : XA0 + QC], in_=blob_d.ap()[:, XA0 : XA0 + QC])
            nc.sync.dma_start(out=blob_t[:, SB0:X3B0], in_=blob_d.ap()[:, SB0:X3B0])
            nc.sync.dma_start(out=blob_t[:, XB0 : XB0 + QC], in_=blob_d.ap()[:, XB0 : XB0 + QC])
            for q in range(1, 8):
                a = XA0 + q * QC
                nc.sync.dma_start(out=blob_t[:, a : a + QC], in_=blob_d.ap()[:, a : a + QC])
                b = XB0 + q * QC
                nc.sync.dma_start(out=blob_t[:, b : b + QC], in_=blob_d.ap()[:, b : b + QC])
            nc.scalar.dma_start(out=blob_t[:, X3B0:BLOB], in_=blob_d.ap()[:, X3B0:BLOB])
            xa_t = blob_t[0 : DH + 1, XA0 : XA0 + H * W]
            xb_t = blob_t[0 : DH + 1, XB0 : XB0 + H * W]
            sa_t = blob_t[0 : DH + 1, SA0 : SA0 + PCOLS]
            sb_t = blob_t[0 : DH + 1, SB0 : SB0 + PCOLS]
            x3b_t = blob_t[:, X3B0 : X3B0 + 3 * HPAD * CIN]
            wcd_t = blob_t[:, WCD0 : WCD0 + NCD * D]
            bias_t = blob_t[:, B0 : B0 + 32 * COUT]

            acc_t = io_pool.tile([128, 32 * COUT], F16)

            xa4 = xa_t.rearrange("k (h w) -> k h w", h=H)
            xb4 = xb_t.rearrange("k (h w) -> k h w", h=H)
            x4 = x3b_t.rearrange("p (kw h c) -> p kw h c", kw=3, h=HPAD)
            w5 = wcd_t.rearrange(
                "p (co kw kh c) -> p co kw kh c", co=NCD, kw=3, kh=3
            )

            def emit_tree(r2, staged):
                s3 = staged[:, :].rearrange("p (u j) -> p u j", j=D)
                t1_t = tr_pool.tile([128, 32 * DH], F16, tag="t1", name=f"t1_{r2}")
                t1 = t1_t[:, :].rearrange("p (u j) -> p u j", j=DH)
                nc.vector.tensor_tensor(
                    out=t1, in0=s3[:, :, 0:DH], in1=s3[:, :, DH:D], op=AluOpType.max
                )
                t2_t = tr_pool.tile([128, 32 * 36], F16, tag="t2", name=f"t2_{r2}")
                t2 = t2_t[:, :].rearrange("p (u j) -> p u j", j=36)
                nc.vector.tensor_tensor(
                    out=t2, in0=t1[:, :, 0:36], in1=t1[:, :, 36:72], op=AluOpType.max
                )
                t3_t = tr_pool.tile([128, 32 * 18], F16, tag="t3", name=f"t3_{r2}")
                t3 = t3_t[:, :].rearrange("p (u j) -> p u j", j=18)
                nc.vector.tensor_tensor(
                    out=t3, in0=t2[:, :, 0:18], in1=t2[:, :, 18:36], op=AluOpType.max
                )
                t4_t = tr_pool.tile([128, 32 * 9], F16, tag="t4", name=f"t4_{r2}")
                t4 = t4_t[:, :].rearrange("p (u j) -> p u j", j=9)
                nc.vector.tensor_tensor(
                    out=t4, in0=t3[:, :, 0:9], in1=t3[:, :, 9:18], op=AluOpType.max
                )
                nc.vector.tensor_reduce(
                    out=acc_t[:, r2 * COUT : (r2 + 1) * COUT],
                    in_=t4,
                    axis=mybir.AxisListType.X,
                    op=AluOpType.max,
                )

            pending = []
            for r2 in range(32):
                r = 2 * r2
                staged = st_pool.tile([128, 32 * D], F16, tag="stg", name=f"stg{r2}")

                for half in range(2):
                    x_t = (xa4 if half == 0 else xb4)[:, r : r + 2, :]
                    s_t = sa_t if half == 0 else sb_t
                    ps_t = ps_pool.tile([128, PCOLS], F32, tag="ps", name=f"ps{r2}_{half}")
                    for m0 in range(0, PCOLS, 512):
                        m1 = min(m0 + 512, PCOLS)
                        nc.tensor.matmul(
                            out=ps_t[:, m0:m1],
                            lhsT=x_t,
                            rhs=s_t[:, m0:m1],
                            start=True,
                            stop=True,
                        )
                    stv = staged[:, :].rearrange("p (u j) -> p u j", j=D)
                    nc.scalar.activation(
                        out=stv[:, 0:NB, half * DH : (half + 1) * DH],
                        in_=ps_t[:, :].rearrange("p (u j) -> p u j", j=DH),
                        func=mybir.ActivationFunctionType.Abs,
                    )

                sc_t = sc_pool.tile([128, NCD * D], F16, tag="sc", name=f"sc{r2}")
                s5 = sc_t[:, :].rearrange(
                    "p (co kw kh c) -> p co kw kh c", co=NCD, kw=3, kh=3
                )
                x5b = (
                    x4[:, :, r : r + 3, :]
                    .unsqueeze(1)
                    .broadcast_to((128, NCD, 3, 3, CIN))
                )
                nc.vector.tensor_tensor(out=s5, in0=x5b, in1=w5, op=AluOpType.subtract)
                if NC:
                    nc.scalar.activation(
                        out=staged[:, NB * D : NB * D + NC * D],
                        in_=sc_t[:, 0 : NC * D],
                        func=mybir.ActivationFunctionType.Abs,
                    )
                if ND:
                    nc.vector.tensor_scalar(
                        out=staged[:, (NB + NC) * D : 32 * D].bitcast(I16),
                        in0=sc_t[:, NC * D : NCD * D].bitcast(I16),
                        scalar1=0x7FFF,
                        scalar2=None,
                        op0=AluOpType.bitwise_and,
                    )

                pending.append((r2, staged))
                if len(pending) > 3:
                    pr2, pst = pending.pop(0)
                    emit_tree(pr2, pst)
                    if pr2 in (7, 15, 23):
                        q = pr2 // 8
                        nc.vector.tensor_tensor(
                            out=acc_t[:, q * 256 : (q + 1) * 256],
                            in0=acc_t[:, q * 256 : (q + 1) * 256],
                            in1=bias_t[:, q * 256 : (q + 1) * 256],
                            op=AluOpType.add,
                        )
                        nc.sync.dma_start(
                            out=out_d.ap()[:, q * 256 : (q + 1) * 256],
                            in_=acc_t[:, q * 256 : (q + 1) * 256],
                        )

            for p in pending:
                emit_tree(*p)

            nc.vector.tensor_tensor(
                out=acc_t[:, 768:1024], in0=acc_t[:, 768:1024],
                in1=bias_t[:, 768:1024], op=AluOpType.add,
            )
            nc.sync.dma_start(out=out_d.ap()[:, 768:1024], in_=acc_t[:, 768:1024])

    nc.compile()
    return nc


def _prep_inputs_hybrid(x, weights, bias):
    NC = 32 - NB - ND
    NCD = NC + ND
    PCOLS = NB * DH
    XA0 = 0
    XB0 = XA0 + H * W
    SA0 = XB0 + H * W
    SB0 = SA0 + PCOLS
    X3B0 = SB0 + PCOLS
    WCD0 = X3B0 + 3 * HPAD * CIN
    B0 = WCD0 + NCD * D
    BLOB = B0 + 32 * COUT

    w_perm = np.ascontiguousarray(weights.transpose(0, 3, 2, 1)).reshape(COUT, D)

    def selector(half):
        s = np.zeros((DH + 1, NB, DH), dtype=np.float32)
        for j in range(DH):
            s[j, :, j] = 1.0
        s[DH, :, :] = -w_perm[:NB, half * DH : (half + 1) * DH]
        return s.reshape(DH + 1, PCOLS).astype(np.float16)

    sa = selector(0)
    sb = selector(1)
    wcd = np.broadcast_to(w_perm[NB:].reshape(1, NCD * D), (128, NCD * D))
    biasb = np.broadcast_to(
        np.tile(bias.reshape(COUT), 32)[None, :], (128, 32 * COUT)
    )

    in_maps = []
    for core in range(N_CORES):
        xc = x[core]
        x_pad = np.pad(xc, ((0, 0), (1, 1), (1, 1)), mode="edge")
        planes = np.empty((3, 3, CIN, H, W), dtype=np.float32)
        for kw in range(3):
            for kh in range(3):
                planes[kw, kh] = x_pad[:, kh : kh + H, kw : kw + W]
        planes = planes.reshape(D, H * W)
        ones = np.ones((1, H * W), dtype=np.float32)
        blob = np.zeros((128, BLOB), dtype=np.float16)
        blob[: DH + 1, XA0 : XA0 + H * W] = np.concatenate([planes[:DH], ones], 0)
        blob[: DH + 1, XB0 : XB0 + H * W] = np.concatenate([planes[DH:], ones], 0)
        blob[: DH + 1, SA0 : SA0 + PCOLS] = sa
        blob[: DH + 1, SB0 : SB0 + PCOLS] = sb
        blob[:, X3B0 : X3B0 + 3 * HPAD * CIN] = _build_x3b_f16(xc)
        blob[:, WCD0 : WCD0 + NCD * D] = wcd
        blob[:, B0 : B0 + 32 * COUT] = biasb
        in_maps.append({"blob": blob})
    return in_maps


def _build_x3b_f16(xc):
    wi = np.clip(np.arange(W)[None, :] + np.arange(-1, 2)[:, None], 0, W - 1)
    halves = []
    for b in range(2):
        h_idx = np.clip(np.arange(HPAD) - 1 + b, 0, H - 1)
        g = xc[:, h_idx, :][:, :, wi]  # (CIN, HPAD, 3, W)
        halves.append(np.ascontiguousarray(g.transpose(3, 2, 1, 0)))
    out = np.stack(halves, axis=0)  # (2, W, 3, HPAD, CIN)
    return np.ascontiguousarray(out.reshape(128, 3 * HPAD * CIN).astype(np.float16))


# ---------------------------------------------------------------- common

def _get_program():
    key = (SCHEME, NB, ND, NB2, L1_ABSMAX)
    if key not in _PROGRAM_CACHE:
        if SCHEME == "v2":
            _PROGRAM_CACHE[key] = _build_program_v2()
        else:
            _PROGRAM_CACHE[key] = _build_program_hybrid()
    return _PROGRAM_CACHE[key]


def _prep_inputs(x, weights, bias):
    if SCHEME == "v2":
        return _prep_inputs_v2(x, weights, bias)
    return _prep_inputs_hybrid(x, weights, bias)


def _unshuffle(o):
    """o: (128, 1024) [(b,w), (r2,co)] -> (COUT, H, W)"""
    return np.ascontiguousarray(
        np.asarray(o).reshape(2, W, 32, COUT).transpose(3, 2, 0, 1).reshape(COUT, H, W)
    )


def kernel(x, weights, bias):
    from concourse.bass_utils import run_bass_kernel_spmd

    global LAST_RESULTS
    nc = _get_program()

    x = np.asarray(x, dtype=np.float32)
    weights = np.asarray(weights, dtype=np.float32)
    bias = np.asarray(bias, dtype=np.float32)

    in_maps = _prep_inputs(x, weights, bias)
    res = run_bass_kernel_spmd(nc, in_maps, core_ids=list(range(N_CORES)))
    LAST_RESULTS = res

    outs = [_unshuffle(res.results[core]["out"]) for core in range(N_CORES)]
    return np.stack(outs).astype(np.float32)


# revision 8
# speedup vs baseline: 1.0143x; 1.0143x over previous
"""Trainium2 Bass kernel for nn_Dist_Conv2D_Dense (Chebyshev-distance "conv").

Computation (per batch b, output channel co, position (h, w)):
    out[b, co, h, w] = max_{cin, kh, kw} |x[b, cin, h+kh-1, w+kw-1] - weights[co, cin, kh, kw]| + bias[co]
with replicate ("edge") padding, for x (8, 16, 64, 64), weights (32, 16, 3, 3).

Sharding: data-parallel over batch, B=8 -> one batch element per NeuronCore.

SCHEME "v2" (default) - 3-engine pipeline, rebalanced + batched tree:
  * TensorE produces (x - w) diffs for the first NB channels via a selector
    matmul (stationary lhsT = 73 rows: 72 pre-shifted input planes + ones row;
    moving columns have a 1 at row d and -w[co,d] in the ones row).
  * ScalarE drains PSUM with Abs, casting to fp16 into a unified staged tile.
  * VectorE subtracts the remaining ND channels directly (fp16 2x broadcast),
    writing RAW diffs into the same staged tile.
  * VectorE max-tree over all 32 channels, batched TWO row-pairs per
    instruction to amortize the ~58-cycle DVE instruction overhead; level 1
    uses op=abs_max which both combines the halves and absorbs the abs for
    the DVE-produced channels (ACT-drained values are nonneg, so abs_max==max).
  * DMA staged so the first matmul's inputs (sel-A chunk 0 + x quarter 0)
    land first, splitting issue across the two HWDGE queues (sync/scalar).

SCHEME "hybrid": previous 141.5us baseline kept for reference/fallback.
"""

import numpy as np
from contextlib import ExitStack

# Problem constants (hardcoded per spec)
B, CIN, H, W = 8, 16, 64, 64
COUT, K = 32, 3
N_CORES = 8
HPAD = H + 2  # 66
D = CIN * K * K  # 144
DH = D // 2  # 72, half-window length

SCHEME = "v2"  # "v2" | "hybrid"
# v2 channel split: NB channels PE->ACT, ND = 32-NB channels DVE-subtract
NB2 = 25
ND2 = 32 - NB2
L1_ABSMAX = False  # abs_max has no TRN2 encoding (walrus rejects); use int16 mask
MASK_GPSIMD = False  # Pool engine rejects TensorScalarPtr (NCC_IXCG966)
# hybrid params (legacy)
NB = 26
ND = 6

_PROGRAM_CACHE = {}
LAST_RESULTS = None  # stashed BassKernelResults for the test harness


# ------------------------------------------------------------------ v2 scheme

def _build_program_v2():
    import concourse.bacc as bacc
    import concourse.mybir as mybir
    from concourse.alu_op_type import AluOpType
    from concourse.tile import TileContext

    F16, F32, I16 = mybir.dt.float16, mybir.dt.float32, mybir.dt.int16
    PCOLS = NB2 * DH  # psum columns per half

    nc = bacc.Bacc(
        "TRN2", target_bir_lowering=False, debug=False, num_devices=N_CORES
    )

    XA0 = 0
    XB0 = XA0 + H * W
    SA0 = XB0 + H * W
    SB0 = SA0 + PCOLS
    X3B0 = SB0 + PCOLS
    WCD0 = X3B0 + 3 * HPAD * CIN
    B0 = WCD0 + ND2 * D
    BLOB = B0 + 32 * COUT
    blob_d = nc.dram_tensor("blob", [128, BLOB], F16, kind="ExternalInput")
    out_d = nc.dram_tensor("out", [128, 32 * COUT], F16, kind="ExternalOutput")

    with TileContext(nc) as tc:
        with (
            tc.tile_pool(name="io", bufs=1) as io_pool,
            tc.tile_pool(name="ps", bufs=2, space="PSUM") as ps_pool,
            tc.tile_pool(name="st", bufs=3) as st_pool,
            tc.tile_pool(name="tr", bufs=2) as tr_pool,
        ):
            blob_t = io_pool.tile([128, BLOB], F16)
            QC = H * W // 8  # 512-column x quarters

            # -- DMA staging, all on the sync queue (a dma_start occupies the
            # issuing engine's NX for ~600ns, so keeping them off scalar/vector
            # protects the ACT/DVE pipelines). Critical-path order: the first
            # A-half matmuls need sel-A chunk 0 + xa quarter 0.
            nc.sync.dma_start(out=blob_t[:, SA0 : SA0 + 512], in_=blob_d.ap()[:, SA0 : SA0 + 512])
            nc.sync.dma_start(out=blob_t[:, XA0 : XA0 + QC], in_=blob_d.ap()[:, XA0 : XA0 + QC])
            nc.sync.dma_start(out=blob_t[:, SB0 : SB0 + 512], in_=blob_d.ap()[:, SB0 : SB0 + 512])
            nc.sync.dma_start(out=blob_t[:, XB0 : XB0 + QC], in_=blob_d.ap()[:, XB0 : XB0 + QC])
            nc.sync.dma_start(out=blob_t[:, SA0 + 512 : SB0], in_=blob_d.ap()[:, SA0 + 512 : SB0])
            nc.sync.dma_start(out=blob_t[:, SB0 + 512 : X3B0], in_=blob_d.ap()[:, SB0 + 512 : X3B0])
            # x3b + wcd: needed by the first DVE subtract (tolerates ~1 rp lag)
            nc.sync.dma_start(out=blob_t[:, X3B0:B0], in_=blob_d.ap()[:, X3B0:B0])
            for q in range(1, 8):
                a = XA0 + q * QC
                nc.sync.dma_start(out=blob_t[:, a : a + QC], in_=blob_d.ap()[:, a : a + QC])
                b = XB0 + q * QC
                nc.sync.dma_start(out=blob_t[:, b : b + QC], in_=blob_d.ap()[:, b : b + QC])
            nc.sync.dma_start(out=blob_t[:, B0:BLOB], in_=blob_d.ap()[:, B0:BLOB])

            xa_t = blob_t[0 : DH + 1, XA0 : XA0 + H * W]
            xb_t = blob_t[0 : DH + 1, XB0 : XB0 + H * W]
            sa_t = blob_t[0 : DH + 1, SA0 : SA0 + PCOLS]
            sb_t = blob_t[0 : DH + 1, SB0 : SB0 + PCOLS]
            x3b_t = blob_t[:, X3B0 : X3B0 + 3 * HPAD * CIN]
            wcd_t = blob_t[:, WCD0 : WCD0 + ND2 * D]
            bias_t = blob_t[:, B0 : B0 + 32 * COUT]

            # acc columns: (r2, co)
            acc_t = io_pool.tile([128, 32 * COUT], F16)

            xa4 = xa_t.rearrange("k (h w) -> k h w", h=H)
            xb4 = xb_t.rearrange("k (h w) -> k h w", h=H)
            x4 = x3b_t.rearrange("p (kw h c) -> p kw h c", kw=3, h=HPAD)
            w5 = wcd_t.rearrange(
                "p (co kw kh c) -> p co kw kh c", co=ND2, kw=3, kh=3
            )

            def produce(r2, staged):
                """Emit PE+ACT+DVE production of row-pair r2 into staged
                (a [128, 2*32*D] tile; r2&1 selects the half)."""
                r = 2 * r2
                sv = staged[:, :].rearrange(
                    "p (e u j) -> p e u j", e=2, j=D
                )
                for half in range(2):
                    x_t = (xa4 if half == 0 else xb4)[:, r : r + 2, :]  # [73,2,64]
                    s_t = sa_t if half == 0 else sb_t
                    ps_t = ps_pool.tile(
                        [128, PCOLS], F32, tag="ps", name=f"ps{r2}_{half}"
                    )
                    for m0 in range(0, PCOLS, 512):
                        m1 = min(m0 + 512, PCOLS)
                        nc.tensor.matmul(
                            out=ps_t[:, m0:m1],
                            lhsT=x_t,
                            rhs=s_t[:, m0:m1],
                            start=True,
                            stop=True,
                        )
                    nc.scalar.activation(
                        out=sv[:, r2 % 2, 0:NB2, half * DH : (half + 1) * DH],
                        in_=ps_t[:, :].rearrange("p (u j) -> p u j", j=DH),
                        func=mybir.ActivationFunctionType.Abs,
                    )
                # DVE subtract for the last ND2 channels (raw diffs; abs is
                # folded into tree L1's abs_max)
                s5 = sv[:, r2 % 2, NB2:32, :].rearrange(
                    "p co (kw kh c) -> p co kw kh c", kw=3, kh=3
                )
                x5b = (
                    x4[:, :, r : r + 3, :]
                    .unsqueeze(1)
                    .broadcast_to((128, ND2, 3, 3, CIN))
                )
                nc.vector.tensor_tensor(out=s5, in0=x5b, in1=w5, op=AluOpType.subtract)
                if not L1_ABSMAX:
                    # abs via int16 sign-strip; on GpSimd (otherwise idle) to
                    # keep the DVE free for the max tree
                    eng = nc.gpsimd if MASK_GPSIMD else nc.vector
                    eng.tensor_scalar(
                        out=sv[:, r2 % 2, NB2:32, :].bitcast(I16),
                        in0=sv[:, r2 % 2, NB2:32, :].bitcast(I16),
                        scalar1=0x7FFF,
                        scalar2=None,
                        op0=AluOpType.bitwise_and,
                    )

            def bias_store(c0, c1):
                nc.vector.tensor_tensor(
                    out=acc_t[:, c0:c1],
                    in0=acc_t[:, c0:c1],
                    in1=bias_t[:, c0:c1],
                    op=AluOpType.add,
                )
                nc.sync.dma_start(
                    out=out_d.ap()[:, c0:c1], in_=acc_t[:, c0:c1]
                )

            def emit_tree(staged, u0, nu, acc0, tag):
                """Max-tree over units [u0, u0+nu) of a staged tile (each unit
                a 144-dim window), writing acc columns [acc0, acc0+nu)."""
                s4 = staged[:, :].rearrange("p (u j) -> p u j", j=D)[:, u0 : u0 + nu, :]
                t1_t = tr_pool.tile([128, nu * DH], F16, tag="t1", name=f"t1_{tag}")
                t1 = t1_t[:, :].rearrange("p (u j) -> p u j", j=DH)
                nc.vector.tensor_tensor(
                    out=t1, in0=s4[:, :, 0:DH], in1=s4[:, :, DH:D], op=AluOpType.max
                )
                t2_t = tr_pool.tile([128, nu * 36], F16, tag="t2", name=f"t2_{tag}")
                t2 = t2_t[:, :].rearrange("p (u j) -> p u j", j=36)
                nc.vector.tensor_tensor(
                    out=t2, in0=t1[:, :, 0:36], in1=t1[:, :, 36:72], op=AluOpType.max
                )
                t3_t = tr_pool.tile([128, nu * 18], F16, tag="t3", name=f"t3_{tag}")
                t3 = t3_t[:, :].rearrange("p (u j) -> p u j", j=18)
                nc.vector.tensor_tensor(
                    out=t3, in0=t2[:, :, 0:18], in1=t2[:, :, 18:36], op=AluOpType.max
                )
                t4_t = tr_pool.tile([128, nu * 9], F16, tag="t4", name=f"t4_{tag}")
                t4 = t4_t[:, :].rearrange("p (u j) -> p u j", j=9)
                nc.vector.tensor_tensor(
                    out=t4, in0=t3[:, :, 0:9], in1=t3[:, :, 9:18], op=AluOpType.max
                )
                nc.vector.tensor_reduce(
                    out=acc_t[:, acc0 : acc0 + nu],
                    in_=t4,
                    axis=mybir.AxisListType.X,
                    op=AluOpType.max,
                )

            staged_tiles = {}
            for r2 in range(32):
                pair = r2 // 2
                if r2 % 2 == 0:
                    staged_tiles[pair] = st_pool.tile(
                        [128, 2 * 32 * D], F16, tag="stg", name=f"stg{pair}"
                    )
                produce(r2, staged_tiles[pair])
                # software pipeline: tree for pair k emitted after pair k+1's
                # production, so ScalarE has a full pair-window to finish.
                # The last pair is de-batched into per-row-pair trees so the
                # final tree only trails the very last drain by one row-pair.
                if r2 % 2 == 1 and 1 <= pair <= 14:
                    emit_tree(staged_tiles.pop(pair - 1), 0, 64, (pair - 1) * 64, pair - 1)
                    if pair - 1 in (3, 7, 11):
                        q = (pair - 1) // 4
                        bias_store(q * 256, (q + 1) * 256)
                elif r2 == 31:
                    emit_tree(staged_tiles[14], 0, 64, 14 * 64, 14)
                    bias_store(768, 960)
            emit_tree(staged_tiles[15], 0, 32, 960, "r30")
            bias_store(960, 992)
            emit_tree(staged_tiles.pop(15), 32, 32, 992, "r31")
            staged_tiles.pop(14)
            bias_store(992, 1024)

    nc.compile()
    return nc


def _prep_inputs_v2(x, weights, bias):
    PCOLS = NB2 * DH
    XA0 = 0
    XB0 = XA0 + H * W
    SA0 = XB0 + H * W
    SB0 = SA0 + PCOLS
    X3B0 = SB0 + PCOLS
    WCD0 = X3B0 + 3 * HPAD * CIN
    B0 = WCD0 + ND2 * D
    BLOB = B0 + 32 * COUT

    w_perm = np.ascontiguousarray(weights.transpose(0, 3, 2, 1)).reshape(COUT, D)

    def selector(half):
        s = np.zeros((DH + 1, NB2, DH), dtype=np.float32)
        for j in range(DH):
            s[j, :, j] = 1.0
        s[DH, :, :] = -w_perm[:NB2, half * DH : (half + 1) * DH]
        return s.reshape(DH + 1, PCOLS).astype(np.float16)

    sa = selector(0)
    sb = selector(1)
    wcd = np.broadcast_to(w_perm[NB2:].reshape(1, ND2 * D), (128, ND2 * D))
    biasb = np.broadcast_to(
        np.tile(bias.reshape(COUT), 32)[None, :], (128, 32 * COUT)
    )

    in_maps = []
    for core in range(N_CORES):
        xc = x[core]
        x_pad = np.pad(xc, ((0, 0), (1, 1), (1, 1)), mode="edge")
        planes = np.empty((3, 3, CIN, H, W), dtype=np.float32)  # (kw, kh, cin, h, w)
        for kw in range(3):
            for kh in range(3):
                planes[kw, kh] = x_pad[:, kh : kh + H, kw : kw + W]
        planes = planes.reshape(D, H * W)
        ones = np.ones((1, H * W), dtype=np.float32)
        blob = np.zeros((128, BLOB), dtype=np.float16)
        blob[: DH + 1, XA0 : XA0 + H * W] = np.concatenate([planes[:DH], ones], 0)
        blob[: DH + 1, XB0 : XB0 + H * W] = np.concatenate([planes[DH:], ones], 0)
        blob[: DH + 1, SA0 : SA0 + PCOLS] = sa
        blob[: DH + 1, SB0 : SB0 + PCOLS] = sb
        blob[:, X3B0 : X3B0 + 3 * HPAD * CIN] = _build_x3b_f16(xc)
        blob[:, WCD0 : WCD0 + ND2 * D] = wcd
        blob[:, B0 : B0 + 32 * COUT] = biasb
        in_maps.append({"blob": blob})
    return in_maps


# ------------------------------------------------------------ hybrid scheme

def _build_program_hybrid():
    import concourse.bacc as bacc
    import concourse.mybir as mybir
    from concourse.alu_op_type import AluOpType
    from concourse.tile import TileContext

    F16, F32, I16 = mybir.dt.float16, mybir.dt.float32, mybir.dt.int16
    NC = 32 - NB - ND          # DVE-sub + ACT-abs channels
    NCD = NC + ND              # all DVE-subtracted channels
    PCOLS = NB * DH            # psum columns per half-chunk

    nc = bacc.Bacc(
        "TRN2", target_bir_lowering=False, debug=False, num_devices=N_CORES
    )

    XA0 = 0
    XB0 = XA0 + H * W
    SA0 = XB0 + H * W
    SB0 = SA0 + PCOLS
    X3B0 = SB0 + PCOLS
    WCD0 = X3B0 + 3 * HPAD * CIN
    B0 = WCD0 + NCD * D
    BLOB = B0 + 32 * COUT
    blob_d = nc.dram_tensor("blob", [128, BLOB], F16, kind="ExternalInput")
    out_d = nc.dram_tensor("out", [128, 32 * COUT], F16, kind="ExternalOutput")

    with TileContext(nc) as tc:
        with (
            tc.tile_pool(name="io", bufs=1) as io_pool,
            tc.tile_pool(name="ps", bufs=2, space="PSUM") as ps_pool,
            tc.tile_pool(name="st", bufs=4) as st_pool,
            tc.tile_pool(name="sc", bufs=6) as sc_pool,
            tc.tile_pool(name="tr", bufs=4) as tr_pool,
        ):
            blob_t = io_pool.tile([128, BLOB], F16)
            nc.sync.dma_start(out=blob_t[:, SA0:SB0], in_=blob_d.ap()[:, SA0:SB0])
            QC = H * W // 8
            nc.sync.dma_start(out=blob_t[:, XA0 : XA0 + QC], in_=blob_d.ap()[:, XA0 : XA0 + QC])
            nc.sync.dma_start(out=blob_t[:, SB0:X3B0], in_=blob_d.ap()[:, SB0:X3B0])
            nc.sync.dma_start(out=blob_t[:, XB0 : XB0 + QC], in_=blob_d.ap()[:, XB0 : XB0 + QC])
            for q in range(1, 8):
                a = XA0 + q * QC
                nc.sync.dma_start(out=blob_t[:, a : a + QC], in_=blob_d.ap()[:, a : a + QC])
                b = XB0 + q * QC
                nc.sync.dma_start(out=blob_t[:, b : b + QC], in_=blob_d.ap()[:, b : b + QC])
            nc.scalar.dma_start(out=blob_t[:, X3B0:BLOB], in_=blob_d.ap()[:, X3B0:BLOB])
            xa_t = blob_t[0 : DH + 1, XA0 : XA0 + H * W]
            xb_t = blob_t[0 : DH + 1, XB0 : XB0 + H * W]
            sa_t = blob_t[0 : DH + 1, SA0 : SA0 + PCOLS]
            sb_t = blob_t[0 : DH + 1, SB0 : SB0 + PCOLS]
            x3b_t = blob_t[:, X3B0 : X3B0 + 3 * HPAD * CIN]
            wcd_t = blob_t[:, WCD0 : WCD0 + NCD * D]
            bias_t = blob_t[:, B0 : B0 + 32 * COUT]

            acc_t = io_pool.tile([128, 32 * COUT], F16)

            xa4 = xa_t.rearrange("k (h w) -> k h w", h=H)
            xb4 = xb_t.rearrange("k (h w) -> k h w", h=H)
            x4 = x3b_t.rearrange("p (kw h c) -> p kw h c", kw=3, h=HPAD)
            w5 = wcd_t.rearrange(
                "p (co kw kh c) -> p co kw kh c", co=NCD, kw=3, kh=3
            )

            def emit_tree(r2, staged):
                s3 = staged[:, :].rearrange("p (u j) -> p u j", j=D)
                t1_t = tr_pool.tile([128, 32 * DH], F16, tag="t1", name=f"t1_{r2}")
                t1 = t1_t[:, :].rearrange("p (u j) -> p u j", j=DH)
                nc.vector.tensor_tensor(
                    out=t1, in0=s3[:, :, 0:DH], in1=s3[:, :, DH:D], op=AluOpType.max
                )
                t2_t = tr_pool.tile([128, 32 * 36], F16, tag="t2", name=f"t2_{r2}")
                t2 = t2_t[:, :].rearrange("p (u j) -> p u j", j=36)
                nc.vector.tensor_tensor(
                    out=t2, in0=t1[:, :, 0:36], in1=t1[:, :, 36:72], op=AluOpType.max
                )
                t3_t = tr_pool.tile([128, 32 * 18], F16, tag="t3", name=f"t3_{r2}")
                t3 = t3_t[:, :].rearrange("p (u j) -> p u j", j=18)
                nc.vector.tensor_tensor(
                    out=t3, in0=t2[:, :, 0:18], in1=t2[:, :, 18:36], op=AluOpType.max
                )
                t4_t = tr_pool.tile([128, 32 * 9], F16, tag="t4", name=f"t4_{r2}")
                t4 = t4_t[:, :].rearrange("p (u j) -> p u j", j=9)
                nc.vector.tensor_tensor(
                    out=t4, in0=t3[:, :, 0:9], in1=t3[:, :, 9:18], op=AluOpType.max
                )
                nc.vector.tensor_reduce(
                    out=acc_t[:, r2 * COUT : (r2 + 1) * COUT],
                    in_=t4,
                    axis=mybir.AxisListType.X,
                    op=AluOpType.max,
                )

            pending = []
            for r2 in range(32):
                r = 2 * r2
                staged = st_pool.tile([128, 32 * D], F16, tag="stg", name=f"stg{r2}")

                for half in range(2):
                    x_t = (xa4 if half == 0 else xb4)[:, r : r + 2, :]
                    s_t = sa_t if half == 0 else sb_t
                    ps_t = ps_pool.tile([128, PCOLS], F32, tag="ps", name=f"ps{r2}_{half}")
                    for m0 in range(0, PCOLS, 512):
                        m1 = min(m0 + 512, PCOLS)
                        nc.tensor.matmul(
                            out=ps_t[:, m0:m1],
                            lhsT=x_t,
                            rhs=s_t[:, m0:m1],
                            start=True,
                            stop=True,
                        )
                    stv = staged[:, :].rearrange("p (u j) -> p u j", j=D)
                    nc.scalar.activation(
                        out=stv[:, 0:NB, half * DH : (half + 1) * DH],
                        in_=ps_t[:, :].rearrange("p (u j) -> p u j", j=DH),
                        func=mybir.ActivationFunctionType.Abs,
                    )

                sc_t = sc_pool.tile([128, NCD * D], F16, tag="sc", name=f"sc{r2}")
                s5 = sc_t[:, :].rearrange(
                    "p (co kw kh c) -> p co kw kh c", co=NCD, kw=3, kh=3
                )
                x5b = (
                    x4[:, :, r : r + 3, :]
                    .unsqueeze(1)
                    .broadcast_to((128, NCD, 3, 3, CIN))
                )
                nc.vector.tensor_tensor(out=s5, in0=x5b, in1=w5, op=AluOpType.subtract)
                if NC:
                    nc.scalar.activation(
                        out=staged[:, NB * D : NB * D + NC * D],
                        in_=sc_t[:, 0 : NC * D],
                        func=mybir.ActivationFunctionType.Abs,
                    )
                if ND:
                    nc.vector.tensor_scalar(
                        out=staged[:, (NB + NC) * D : 32 * D].bitcast(I16),
                        in0=sc_t[:, NC * D : NCD * D].bitcast(I16),
                        scalar1=0x7FFF,
                        scalar2=None,
                        op0=AluOpType.bitwise_and,
                    )

                pending.append((r2, staged))
                if len(pending) > 3:
                    pr2, pst = pending.pop(0)
                    emit_tree(pr2, pst)
                    if pr2 in (7, 15, 23):
                        q = pr2 // 8
                        nc.vector.tensor_tensor(
                            out=acc_t[:, q * 256 : (q + 1) * 256],
                            in0=acc_t[:, q * 256 : (q + 1) * 256],
                            in1=bias_t[:, q * 256 : (q + 1) * 256],
                            op=AluOpType.add,
                        )
                        nc.sync.dma_start(
                            out=out_d.ap()[:, q * 256 : (q + 1) * 256],
                            in_=acc_t[:, q * 256 : (q + 1) * 256],
                        )

            for p in pending:
                emit_tree(*p)

            nc.vector.tensor_tensor(
                out=acc_t[:, 768:1024], in0=acc_t[:, 768:1024],
                in1=bias_t[:, 768:1024], op=AluOpType.add,
            )
            nc.sync.dma_start(out=out_d.ap()[:, 768:1024], in_=acc_t[:, 768:1024])

    nc.compile()
    return nc


def _prep_inputs_hybrid(x, weights, bias):
    NC = 32 - NB - ND
    NCD = NC + ND
    PCOLS = NB * DH
    XA0 = 0
    XB0 = XA0 + H * W
    SA0 = XB0 + H * W
    SB0 = SA0 + PCOLS
    X3B0 = SB0 + PCOLS
    WCD0 = X3B0 + 3 * HPAD * CIN
    B0 = WCD0 + NCD * D
    BLOB = B0 + 32 * COUT

    w_perm = np.ascontiguousarray(weights.transpose(0, 3, 2, 1)).reshape(COUT, D)

    def selector(half):
        s = np.zeros((DH + 1, NB, DH), dtype=np.float32)
        for j in range(DH):
            s[j, :, j] = 1.0
        s[DH, :, :] = -w_perm[:NB, half * DH : (half + 1) * DH]
        return s.reshape(DH + 1, PCOLS).astype(np.float16)

    sa = selector(0)
    sb = selector(1)
    wcd = np.broadcast_to(w_perm[NB:].reshape(1, NCD * D), (128, NCD * D))
    biasb = np.broadcast_to(
        np.tile(bias.reshape(COUT), 32)[None, :], (128, 32 * COUT)
    )

    in_maps = []
    for core in range(N_CORES):
        xc = x[core]
        x_pad = np.pad(xc, ((0, 0), (1, 1), (1, 1)), mode="edge")
        planes = np.empty((3, 3, CIN, H, W), dtype=np.float32)
        for kw in range(3):
            for kh in range(3):
                planes[kw, kh] = x_pad[:, kh : kh + H, kw : kw + W]
        planes = planes.reshape(D, H * W)
        ones = np.ones((1, H * W), dtype=np.float32)
        blob = np.zeros((128, BLOB), dtype=np.float16)
        blob[: DH + 1, XA0 : XA0 + H * W] = np.concatenate([planes[:DH], ones], 0)
        blob[: DH + 1, XB0 : XB0 + H * W] = np.concatenate([planes[DH:], ones], 0)
        blob[: DH + 1, SA0 : SA0 + PCOLS] = sa
        blob[: DH + 1, SB0 : SB0 + PCOLS] = sb
        blob[:, X3B0 : X3B0 + 3 * HPAD * CIN] = _build_x3b_f16(xc)
        blob[:, WCD0 : WCD0 + NCD * D] = wcd
        blob[:, B0 : B0 + 32 * COUT] = biasb
        in_maps.append({"blob": blob})
    return in_maps


def _build_x3b_f16(xc):
    wi = np.clip(np.arange(W)[None, :] + np.arange(-1, 2)[:, None], 0, W - 1)
    halves = []
    for b in range(2):
        h_idx = np.clip(np.arange(HPAD) - 1 + b, 0, H - 1)
        g = xc[:, h_idx, :][:, :, wi]  # (CIN, HPAD, 3, W)
        halves.append(np.ascontiguousarray(g.transpose(3, 2, 1, 0)))
    out = np.stack(halves, axis=0)  # (2, W, 3, HPAD, CIN)
    return np.ascontiguousarray(out.reshape(128, 3 * HPAD * CIN).astype(np.float16))


# ---------------------------------------------------------------- common

def _get_program():
    key = (SCHEME, NB, ND, NB2, L1_ABSMAX)
    if key not in _PROGRAM_CACHE:
        if SCHEME == "v2":
            _PROGRAM_CACHE[key] = _build_program_v2()
        else:
            _PROGRAM_CACHE[key] = _build_program_hybrid()
    return _PROGRAM_CACHE[key]


def _prep_inputs(x, weights, bias):
    if SCHEME == "v2":
        return _prep_inputs_v2(x, weights, bias)
    return _prep_inputs_hybrid(x, weights, bias)


def _unshuffle(o):
    """o: (128, 1024) [(b,w), (r2,co)] -> (COUT, H, W)"""
    return np.ascontiguousarray(
        np.asarray(o).reshape(2, W, 32, COUT).transpose(3, 2, 0, 1).reshape(COUT, H, W)
    )


def kernel(x, weights, bias):
    from concourse.bass_utils import run_bass_kernel_spmd

    global LAST_RESULTS
    nc = _get_program()

    x = np.asarray(x, dtype=np.float32)
    weights = np.asarray(weights, dtype=np.float32)
    bias = np.asarray(bias, dtype=np.float32)

    in_maps = _prep_inputs(x, weights, bias)
    res = run_bass_kernel_spmd(nc, in_maps, core_ids=list(range(N_CORES)))
    LAST_RESULTS = res

    outs = [_unshuffle(res.results[core]["out"]) for core in range(N_CORES)]
    return np.stack(outs).astype(np.float32)
